# revision 20
# baseline (speedup 1.0000x reference)
"""Trainium2 Bass kernel for nn_BatchGraphEncoder (gnn_message_passing).

Math note: the reference's segment softmax uses B unique segment ids
(groups of size 1), so alpha == exp(x-x)/1 == 1.0 bit-exactly for any
finite scores.  The output is therefore independent of the attention
inputs (w_i, w_j, w_k) and reduces to pure batch sums:

    out[:,   0:128] = sum_b h[b,:]      (broadcast over the N=512 rows)
    out[:, 128:256] = sum_b r[b,:]      (broadcast)
    out[:, 256:384] = sum_b t[b,:,:]    ([512, 128])

This is a memory-bound reduction over B=2048 dominated by reading t
(512 MB).  Strategy: shard B across the 8 cores (data parallel), reduce
over the local batch on-device, and sum the 8 tiny partials on the host.

Pipeline design (v2): the previous per-tile fold-tree (3 halving folds
+ accumulator merge, ~10.5us of DVE per 4 MB tile) slot-coupled the DMA
stream to the DVE and collapsed the last ~40us of the stream to fold
pace.  Now each tile gets ONE DVE op: tensor_tensor add into a wide
[128, 4096] accumulator (free dim packs 8 batch-row slots x 512 cols).
2 MB tiles / 10 pool buffers keep the DVE ~20% faster than the DMA
stream with a 20 MB elasticity window; a 1 MB-tile tail drains the
backlog, and one 4096->512 fold at the very end produces the partial.

Tile layout: partition p holds flat columns [512p, 512p+512) of the
[B_loc, 65536] shard; the free dim packs NB batch rows.  DMA issue
alternates between the SP and ACT HWDGE rings.

The h/r sums ride on the otherwise-idle TensorEngine: a stationary
matrix whose column j is all-ones places column-sums of the moving
operand into PSUM row j (rows 0/1 = sum_h/sum_r).

Load balancing: cores 4 and 6 of this machine usually lose ~10% DMA
bandwidth (one slow SDMA engine each), so they get smaller shards:
rows [224, 240) are only loaded when partition_id != 6, rows [240, 264)
when partition_id not in {4, 6} (the skipping cores' buffers hold stale
finite data there; a per-partition scalar mask gates those tiles'
accumulator merges, and h/r padding rows are zeros, which is exact).
"""

import numpy as np

B, N, D = 2048, 512, 128
NCORES = 8
FLAT = N * D                 # 65536 flattened (n, d) columns
MMW = 512                    # columns per row-slot in the free dim
ACCW = 8 * MMW               # accumulator free width (8 row slots)

# Clean-run per-row rates: fast cores 0.727 us/row, core 4 0.773,
# core 6 0.823 (cores 4/6 lose ~6%/12% DMA bandwidth consistently).
# Equalized shards: rows [230, 246) are skipped on core 6, rows
# [246, 262) on cores 4 and 6.
B_FAST = 262
SIZES = [B_FAST] * NCORES
SIZES[4] = 246
SIZES[6] = 230
assert sum(SIZES) == B

# (row0, NB, conditional) in emission order.  The stream is shaped so
# the in-order DVE merge chain never trails the stream end: conditional
# big tiles sit right after the 10-tile slot warm-up (their slots are
# initialized, and their 4.5us masked merges land early), the remaining
# big tiles finish ~30us before the stream does, and the tail tapers to
# 1 MB then 0.5 MB tiles whose merges (2.3us / 1.2us) sit below their
# DMA times so the DVE rides the stream out with zero lag.  Staged
# pre-folds (slots 4-7 during the NB=4 tiles, slots 2-3 during the NB=2
# tiles) leave only a ~2us fold chain after the last input byte.
_UA = [(r, 16, None) for r in range(0, 80, 16)]      # 5 warm-up 4 MB tiles
_C6 = [(230, 8, "c6"), (238, 8, "c6")]
_C46 = [(246, 8, "c46"), (254, 8, "c46")]
_UB = [(r, 16, None) for r in range(80, 176, 16)]    # 6 big 4 MB tiles
_T8 = [(r, 8, None) for r in range(176, 200, 8)]     # 3 x 2 MB
TILE_PLAN = _UA + _C6 + _C46 + _UB + _T8
assert sum(nb for _, nb, _ in TILE_PLAN) == B_FAST - 30
assert sorted(r for r, nb, c in TILE_PLAN) == sorted(
    r for r, nb, c in _UA + _C6 + _C46 + _UB + _T8
)
# The 30-row taper is carved out of two further pool slots as sub-range
# DMAs (4 x NB=4 quarters, then 7 x NB=2 eighths): sub-DMAs into
# disjoint ranges of one slot don't gate on each other's merges, so the
# stream end can't starve on the pool window, while merge granularity
# tapers to 0.5 MB.
_T4 = [(r, 4) for r in range(200, 216, 4)]
_T2 = [(r, 2) for r in range(216, 230, 2)]

NBUFS = 5

_BUILT = None
# test.py can inject {"trace": True, ...} here; harness path leaves it empty.
RUN_KWARGS = {}
LAST_RESULTS = None


def _build():
    from concourse import bacc, tile, mybir

    f32 = mybir.dt.float32
    add = mybir.AluOpType.add
    nc = bacc.Bacc(
        "TRN2",
        target_bir_lowering=False,
        debug=False,
        enable_asserts=False,
        num_devices=NCORES,
    )
    t_in = nc.dram_tensor("t_shard", [B_FAST, FLAT], f32, kind="ExternalInput").ap()
    h_in = nc.dram_tensor("h_shard", [B_FAST, D], f32, kind="ExternalInput").ap()
    r_in = nc.dram_tensor("r_shard", [B_FAST, D], f32, kind="ExternalInput").ap()
    out_t = nc.dram_tensor("out_t_part", [128, MMW], f32, kind="ExternalOutput").ap()
    out_hr = nc.dram_tensor("out_hr_part", [2, D], f32, kind="ExternalOutput").ap()

    with tile.TileContext(nc) as tc:
        with (
            tc.tile_pool(name="wconst", bufs=1) as wpool,
            tc.tile_pool(name="loads", bufs=NBUFS) as loads,
            tc.tile_pool(name="hr", bufs=6) as hrpool,
            tc.tile_pool(name="res", bufs=1) as res,
            tc.tile_pool(name="acc", bufs=1, space="PSUM") as ppool,
        ):
            W = wpool.tile([128, 256], f32)
            mask6 = wpool.tile([128, 1], f32)
            mask46 = wpool.tile([128, 1], f32)
            psum_hr = ppool.tile([128, D], f32)
            acc = res.tile([128, ACCW], f32)
            res_hr = res.tile([2, D], f32)
            skip_cond = {}
            masks = {"c6": mask6, "c46": mask46}

            def emit_setup_and_hr():
                # Emitted after the first few t loads so the pid register
                # loads and h/r DMAs never delay the t stream's start; h/r
                # loads ride the SWDGE (gpsimd) ring, keeping both HWDGE
                # rings exclusively on t tiles.
                # W is zero except column 128 == 1.0; W[:, 128-j : 256-j]
                # is a [128, 128] stationary whose column j is all-ones.
                nc.vector.memset(W[:], 0.0)
                nc.vector.memset(W[:, 128:129], 1.0)
                # mask6/mask46 = 0.0 on the core(s) that skip that tier,
                # 1.0 elsewhere; they gate the accumulator merges of the
                # conditional tiles.
                nc.vector.memset(mask6[:], 1.0)
                nc.vector.memset(mask46[:], 1.0)
                pid_vec = nc.vector.partition_id()
                with tc.If(pid_vec == 6):
                    nc.vector.memset(mask6[:], 0.0)
                    nc.vector.memset(mask46[:], 0.0)
                with tc.If(pid_vec == 4):
                    nc.vector.memset(mask46[:], 0.0)
                pid_sync = nc.sync.partition_id()
                pid_act = nc.scalar.partition_id()
                skip_cond["c6"] = {
                    nc.sync: pid_sync != 6,
                    nc.scalar: pid_act != 6,
                }
                skip_cond["c46"] = {
                    nc.sync: (pid_sync != 6) * (pid_sync != 4),
                    nc.scalar: (pid_act != 6) * (pid_act != 4),
                }

                # h / r batch sums -> rows 0 / 1 of psum_hr
                # (padding rows on short-shard cores are zeros; exact)
                chunks = []
                for row, src in ((0, h_in), (1, r_in)):
                    for c0 in range(0, B_FAST, 128):
                        k = min(128, B_FAST - c0)
                        ht = hrpool.tile([128, D], f32)
                        nc.gpsimd.dma_start(ht[:k, :], src[c0 : c0 + k, :])
                        chunks.append((row, ht, k))
                for i, (row, ht, k) in enumerate(chunks):
                    nc.tensor.matmul(
                        psum_hr[:],
                        W[:k, 128 - row : 256 - row],
                        ht[:k, :],
                        start=(i == 0),
                        stop=(i == len(chunks) - 1),
                    )
                # Ship the h/r partial mid-stream, off the tail.
                nc.vector.tensor_copy(res_hr[:], psum_hr[0:2, :])
                nc.sync.dma_start(out_hr[:], res_hr[:])

            # --- t batch sum: one DVE merge per tile into acc ---
            ring_bytes = [0, 0]  # greedy byte-balance across the 2 HWDGE rings

            def pick_ring(k, nb):
                ring = (
                    (k % 2)
                    if ring_bytes[0] == ring_bytes[1]
                    else int(ring_bytes[1] < ring_bytes[0])
                )
                ring_bytes[ring] += nb
                return nc.sync if ring == 0 else nc.scalar

            def emit_dma(tl, off, b0, NB, cnd, k):
                fw = NB * MMW
                src = t_in[b0 : b0 + NB, :].rearrange("b (p c) -> p b c", p=128)
                dma = pick_ring(k, NB)
                dst = tl[:, off : off + fw].rearrange("p (b c) -> p b c", b=NB)
                if cnd:
                    # Skipped on the slow core(s): the slot then holds stale
                    # (finite) data from an earlier tile; the masked merge
                    # zeroes it.
                    dma.dma_start(dst, src, cond=skip_cond[cnd][dma])
                else:
                    dma.dma_start(dst, src)

            # Every multi-MB tile is issued as sub-range DMAs so merge
            # completions arrive at <= 2 MB granularity: the first tile
            # as 4 x 1 MB (the DVE's first merge can start ~13us in
            # instead of waiting a whole 4 MB ring drain), NB=16 tiles
            # as 2 MB halves, conditional tiles as 2 masked halves.
            for k, (b0, NB, cnd) in enumerate(TILE_PLAN):
                if k == 2:
                    emit_setup_and_hr()
                fw = NB * MMW  # free width
                tl = loads.tile([128, 16 * MMW], f32, tag="tload")
                if k == 0:
                    for i in range(4):
                        emit_dma(tl, i * 2048, b0 + 4 * i, 4, None, i)
                    half = ACCW // 2
                    nc.vector.tensor_copy(acc[:, :half], tl[:, :half])
                    nc.vector.tensor_copy(acc[:, half:ACCW], tl[:, half : 2 * half])
                    nc.vector.tensor_tensor(
                        acc[:, :half], acc[:, :half], tl[:, 2 * half : 3 * half], add
                    )
                    nc.vector.tensor_tensor(
                        acc[:, half:ACCW], acc[:, half:ACCW], tl[:, 3 * half :], add
                    )
                elif cnd:
                    # two masked half-merges: acc = (sub * mask) + acc
                    half = fw // 2
                    emit_dma(tl, 0, b0, NB // 2, cnd, k)
                    emit_dma(tl, half, b0 + NB // 2, NB // 2, cnd, k + 1)
                    for c0 in (0, half):
                        nc.vector.scalar_tensor_tensor(
                            acc[:, c0 : c0 + half],
                            tl[:, c0 : c0 + half],
                            masks[cnd][:],
                            acc[:, c0 : c0 + half],
                            mybir.AluOpType.mult,
                            add,
                        )
                elif NB == 16:
                    # two 2 MB sub-DMAs, two ACCW-wide merges
                    emit_dma(tl, 0, b0, 8, None, k)
                    emit_dma(tl, ACCW, b0 + 8, 8, None, k + 1)
                    nc.vector.tensor_tensor(acc[:], acc[:], tl[:, :ACCW], add)
                    nc.vector.tensor_tensor(
                        acc[:], acc[:], tl[:, ACCW : 2 * ACCW], add
                    )
                else:
                    emit_dma(tl, 0, b0, NB, cnd, k)
                    nc.vector.tensor_tensor(acc[:, :fw], acc[:, :fw], tl[:, :fw], add)

            # --- taper: sub-range DMAs into two further pool slots ---
            k = len(TILE_PLAN)
            tla = loads.tile([128, 16 * MMW], f32, tag="tload")
            for i, (b0, NB) in enumerate(_T4):
                emit_dma(tla, i * 4 * MMW, b0, NB, None, k + i)
            tlb = loads.tile([128, 16 * MMW], f32, tag="tload")
            for i, (b0, NB) in enumerate(_T2):
                emit_dma(tlb, i * 2 * MMW, b0, NB, None, k + 4 + i)

            # slots 4-7 of acc are final; fold while the taper streams:
            # cols [2048, 3072) := s4+s6 | s5+s7
            nc.vector.tensor_tensor(
                acc[:, 2048:3072], acc[:, 2048:3072], acc[:, 3072:4096], add
            )
            for i in range(len(_T4)):
                o = i * 4 * MMW
                nc.vector.tensor_tensor(
                    acc[:, :2048], acc[:, :2048], tla[:, o : o + 2048], add
                )
            # slots 2-3 final; fold in s4..s7 while the NB=2 subs stream:
            # cols [1024, 2048) := s2+s4+s6 | s3+s5+s7
            nc.vector.tensor_tensor(
                acc[:, 1024:2048], acc[:, 1024:2048], acc[:, 2048:3072], add
            )
            for i in range(len(_T2)):
                o = i * 2 * MMW
                nc.vector.tensor_tensor(
                    acc[:, :1024], acc[:, :1024], tlb[:, o : o + 1024], add
                )

            # Final fold chain after the last merge: 1024 elems, then the
            # last 512 in two halves so each half's output DMA overlaps
            # the other half's fold.
            nc.vector.tensor_tensor(
                acc[:, :1024], acc[:, :1024], acc[:, 1024:2048], add
            )
            nc.vector.tensor_tensor(acc[:, :256], acc[:, :256], acc[:, 512:768], add)
            nc.sync.dma_start(out_t[:, :256], acc[:, :256])
            nc.vector.tensor_tensor(
                acc[:, 256:512], acc[:, 256:512], acc[:, 768:1024], add
            )
            nc.scalar.dma_start(out_t[:, 256:], acc[:, 256:512])

    nc.compile()
    return nc


def _get_built():
    global _BUILT
    if _BUILT is None:
        _BUILT = _build()
    return _BUILT


def kernel(h, r, t, w_i, w_j, w_k):
    global LAST_RESULTS
    from concourse import bass_utils

    nc = _get_built()
    t2 = np.ascontiguousarray(t, dtype=np.float32).reshape(B, FLAT)
    h = np.ascontiguousarray(h, dtype=np.float32)
    r = np.ascontiguousarray(r, dtype=np.float32)

    def pad(a, ncols):
        out = np.zeros((B_FAST, ncols), dtype=np.float32)
        out[: a.shape[0]] = a
        return out

    starts = np.concatenate([[0], np.cumsum(SIZES)])
    in_maps = []
    for c in range(NCORES):
        s, e = int(starts[c]), int(starts[c + 1])
        if e - s == B_FAST:
            in_maps.append({"t_shard": t2[s:e], "h_shard": h[s:e], "r_shard": r[s:e]})
        else:
            in_maps.append(
                {
                    "t_shard": pad(t2[s:e], FLAT),
                    "h_shard": pad(h[s:e], D),
                    "r_shard": pad(r[s:e], D),
                }
            )
    results = bass_utils.run_bass_kernel_spmd(
        nc, in_maps, core_ids=list(range(NCORES)), **RUN_KWARGS
    )
    LAST_RESULTS = results

    sum_t = np.zeros(FLAT, dtype=np.float64)
    sum_h = np.zeros(D, dtype=np.float64)
    sum_r = np.zeros(D, dtype=np.float64)
    for c in range(NCORES):
        sum_t += results.results[c]["out_t_part"].reshape(FLAT)
        sum_h += results.results[c]["out_hr_part"][0]
        sum_r += results.results[c]["out_hr_part"][1]

    out = np.empty((N, 3 * D), dtype=np.float32)
    out[:, 0:D] = sum_h.astype(np.float32)[None, :]
    out[:, D : 2 * D] = sum_r.astype(np.float32)[None, :]
    out[:, 2 * D :] = sum_t.astype(np.float32).reshape(N, D)
    return out


# revision 21
# speedup vs baseline: 1.0406x; 1.0406x over previous
"""Trainium2 Bass kernel for nn_BatchGraphEncoder (gnn_message_passing).

Math note: the reference's segment softmax uses B unique segment ids
(groups of size 1), so alpha == exp(x-x)/1 == 1.0 bit-exactly for any
finite scores.  The output is therefore independent of the attention
inputs (w_i, w_j, w_k) and reduces to pure batch sums:

    out[:,   0:128] = sum_b h[b,:]      (broadcast over the N=512 rows)
    out[:, 128:256] = sum_b r[b,:]      (broadcast)
    out[:, 256:384] = sum_b t[b,:,:]    ([512, 128])

This is a memory-bound reduction over B=2048 dominated by reading t
(512 MB).  Strategy: shard B across the 8 cores (data parallel), reduce
over the local batch on-device, and sum the 8 tiny partials on the host.

Layout (8 KB descriptors): partition p = 32*(b%4) + q holds flat
columns [2048q, 2048q+2048) of batch rows congruent to b%4; each DMA
descriptor moves one 8 KB contiguous run (vs 2 KB when one row spans
all 128 partitions), cutting per-descriptor SDMA overhead.  The DVE
merges tiles into a [128, 4096] accumulator (two 2048-wide bh slots);
the final partial ships as [128, 2048] and the host folds the 4
partition groups (b%4) along with the cross-core sum.

Pipeline: one tensor_tensor merge per 2 MB sub-DMA, 5 x 4 MB pool
slots, the first tile split 4 x 1 MB so the DVE starts ~13us in, and a
1 MB-granularity taper carved from two pool slots as sub-range DMAs so
the stream end never starves on the pool window.

The h/r sums ride on the otherwise-idle TensorEngine: a stationary
matrix whose column j is all-ones places column-sums of the moving
operand into PSUM row j (rows 0/1 = sum_h/sum_r).

Load balancing: cores 4 and 6 of this machine lose ~6%/12% DMA
bandwidth, so they get smaller shards: rows [224, 240) are only loaded
when partition_id != 6, rows [240, 264) when partition_id not in
{4, 6} (the skipping cores' buffers hold stale finite data there; a
per-partition scalar mask gates those tiles' accumulator merges, and
h/r padding rows are zeros, which is exact for a sum).
"""

import numpy as np

B, N, D = 2048, 512, 128
NCORES = 8
FLAT = N * D                 # 65536 flattened (n, d) columns
BLK = 2048                   # flat columns per partition block (8 KB runs)
ACCW = 2 * BLK               # accumulator free width (2 bh slots)

B_FAST = 264
SIZES = [B_FAST] * NCORES
SIZES[4] = 240
SIZES[6] = 224
assert sum(SIZES) == B

_UA = [(r, 16, None) for r in range(0, 80, 16)]      # 5 warm-up 4 MB tiles
_C6 = [(224, 8, "c6"), (232, 8, "c6")]
_C46 = [(240, 8, "c46"), (248, 8, "c46"), (256, 8, "c46")]
_UB = [(r, 16, None) for r in range(80, 176, 16)]    # 6 big 4 MB tiles
_T8 = [(r, 8, None) for r in range(176, 200, 8)]     # 3 x 2 MB
TILE_PLAN = _UA + _C6 + _C46 + _UB + _T8
assert sum(nb for _, nb, _ in TILE_PLAN) == B_FAST - 24
# 1 MB-granularity taper carved from two further pool slots.
_TA = [(r, 4) for r in range(200, 216, 4)]           # 4 subs in slot A
_TB = [(r, 4) for r in range(216, 224, 4)]           # 2 subs in slot B

NBUFS = 5

_BUILT = None
# test.py can inject {"trace": True, ...} here; harness path leaves it empty.
RUN_KWARGS = {}
LAST_RESULTS = None


def _build():
    from concourse import bacc, tile, mybir

    f32 = mybir.dt.float32
    add = mybir.AluOpType.add
    nc = bacc.Bacc(
        "TRN2",
        target_bir_lowering=False,
        debug=False,
        enable_asserts=False,
        num_devices=NCORES,
    )
    t_in = nc.dram_tensor("t_shard", [B_FAST, FLAT], f32, kind="ExternalInput").ap()
    h_in = nc.dram_tensor("h_shard", [B_FAST, D], f32, kind="ExternalInput").ap()
    r_in = nc.dram_tensor("r_shard", [B_FAST, D], f32, kind="ExternalInput").ap()
    out_t = nc.dram_tensor("out_t_part", [128, BLK], f32, kind="ExternalOutput").ap()
    out_hr = nc.dram_tensor("out_hr_part", [2, D], f32, kind="ExternalOutput").ap()

    with tile.TileContext(nc) as tc:
        with (
            tc.tile_pool(name="wconst", bufs=1) as wpool,
            tc.tile_pool(name="loads", bufs=NBUFS) as loads,
            tc.tile_pool(name="hr", bufs=6) as hrpool,
            tc.tile_pool(name="res", bufs=1) as res,
            tc.tile_pool(name="acc", bufs=1, space="PSUM") as ppool,
        ):
            W = wpool.tile([128, 256], f32)
            mask6 = wpool.tile([128, 1], f32)
            mask46 = wpool.tile([128, 1], f32)
            psum_hr = ppool.tile([128, D], f32)
            acc = res.tile([128, ACCW], f32)
            res_hr = res.tile([2, D], f32)
            skip_cond = {}
            masks = {"c6": mask6, "c46": mask46}

            def emit_setup_and_hr():
                nc.vector.memset(W[:], 0.0)
                nc.vector.memset(W[:, 128:129], 1.0)
                nc.vector.memset(mask6[:], 1.0)
                nc.vector.memset(mask46[:], 1.0)
                pid_vec = nc.vector.partition_id()
                with tc.If(pid_vec == 6):
                    nc.vector.memset(mask6[:], 0.0)
                    nc.vector.memset(mask46[:], 0.0)
                with tc.If(pid_vec == 4):
                    nc.vector.memset(mask46[:], 0.0)
                pid_sync = nc.sync.partition_id()
                pid_act = nc.scalar.partition_id()
                skip_cond["c6"] = {
                    nc.sync: pid_sync != 6,
                    nc.scalar: pid_act != 6,
                }
                skip_cond["c46"] = {
                    nc.sync: (pid_sync != 6) * (pid_sync != 4),
                    nc.scalar: (pid_act != 6) * (pid_act != 4),
                }
                chunks = []
                for row, src in ((0, h_in), (1, r_in)):
                    for c0 in range(0, B_FAST, 128):
                        k = min(128, B_FAST - c0)
                        ht = hrpool.tile([128, D], f32)
                        nc.gpsimd.dma_start(ht[:k, :], src[c0 : c0 + k, :])
                        chunks.append((row, ht, k))
                for i, (row, ht, k) in enumerate(chunks):
                    nc.tensor.matmul(
                        psum_hr[:],
                        W[:k, 128 - row : 256 - row],
                        ht[:k, :],
                        start=(i == 0),
                        stop=(i == len(chunks) - 1),
                    )
                nc.vector.tensor_copy(res_hr[:], psum_hr[0:2, :])
                nc.sync.dma_start(out_hr[:], res_hr[:])

            ring_bytes = [0, 0]  # greedy byte-balance across the 2 HWDGE rings

            def pick_ring(k, nb):
                ring = (
                    (k % 2)
                    if ring_bytes[0] == ring_bytes[1]
                    else int(ring_bytes[1] < ring_bytes[0])
                )
                ring_bytes[ring] += nb
                return nc.sync if ring == 0 else nc.scalar

            def emit_dma(tl, off, b0, NB, cnd, k):
                # partition (b%4, q); 8 KB contiguous runs per descriptor
                fw = NB * 512
                src = t_in[b0 : b0 + NB, :].rearrange(
                    "(bh bl) (q c) -> (bl q) bh c", bl=4, c=BLK
                )
                dma = pick_ring(k, NB)
                dst = tl[:, off : off + fw].rearrange("p (b c) -> p b c", b=NB // 4)
                if cnd:
                    dma.dma_start(dst, src, cond=skip_cond[cnd][dma])
                else:
                    dma.dma_start(dst, src)

            for k, (b0, NB, cnd) in enumerate(TILE_PLAN):
                if k == 2:
                    emit_setup_and_hr()
                fw = NB * 512  # free width
                tl = loads.tile([128, 8192], f32, tag="tload")
                if k == 0:
                    for i in range(4):
                        emit_dma(tl, i * BLK, b0 + 4 * i, 4, None, i)
                    nc.vector.tensor_copy(acc[:, :BLK], tl[:, :BLK])
                    nc.vector.tensor_copy(acc[:, BLK:ACCW], tl[:, BLK : 2 * BLK])
                    nc.vector.tensor_tensor(
                        acc[:, :BLK], acc[:, :BLK], tl[:, 2 * BLK : 3 * BLK], add
                    )
                    nc.vector.tensor_tensor(
                        acc[:, BLK:ACCW], acc[:, BLK:ACCW], tl[:, 3 * BLK :], add
                    )
                elif cnd:
                    # two masked half-merges: acc = (sub * mask) + acc
                    half = fw // 2
                    emit_dma(tl, 0, b0, NB // 2, cnd, k)
                    emit_dma(tl, half, b0 + NB // 2, NB // 2, cnd, k + 1)
                    for c0 in (0, half):
                        nc.vector.scalar_tensor_tensor(
                            acc[:, c0 : c0 + half],
                            tl[:, c0 : c0 + half],
                            masks[cnd][:],
                            acc[:, c0 : c0 + half],
                            mybir.AluOpType.mult,
                            add,
                        )
                elif NB == 16:
                    emit_dma(tl, 0, b0, 8, None, k)
                    emit_dma(tl, ACCW, b0 + 8, 8, None, k + 1)
                    nc.vector.tensor_tensor(acc[:], acc[:], tl[:, :ACCW], add)
                    nc.vector.tensor_tensor(
                        acc[:], acc[:], tl[:, ACCW : 2 * ACCW], add
                    )
                else:
                    emit_dma(tl, 0, b0, NB, cnd, k)
                    nc.vector.tensor_tensor(acc[:, :fw], acc[:, :fw], tl[:, :fw], add)

            # --- taper: 1 MB sub-range DMAs into two further pool slots ---
            k = len(TILE_PLAN)
            tla = loads.tile([128, 8192], f32, tag="tload")
            for i, (b0, NB) in enumerate(_TA):
                emit_dma(tla, i * BLK, b0, NB, None, k + i)
            tlb = loads.tile([128, 8192], f32, tag="tload")
            for i, (b0, NB) in enumerate(_TB):
                emit_dma(tlb, i * BLK, b0, NB, None, k + 4 + i)

            # bh slot 1 is final after the last NB>=8 tile; fold it while
            # the taper streams, then merge taper subs into slot 0.
            nc.vector.tensor_tensor(acc[:, :BLK], acc[:, :BLK], acc[:, BLK:ACCW], add)
            for i in range(len(_TA)):
                o = i * BLK
                nc.vector.tensor_tensor(
                    acc[:, :BLK], acc[:, :BLK], tla[:, o : o + BLK], add
                )
            for i in range(len(_TB)):
                o = i * BLK
                nc.vector.tensor_tensor(
                    acc[:, :BLK], acc[:, :BLK], tlb[:, o : o + BLK], add
                )

            # Ship the [128, 2048] partial in two ring-overlapped halves;
            # the host folds the four b%4 partition groups.
            nc.sync.dma_start(out_t[:, :BLK // 2], acc[:, : BLK // 2])
            nc.scalar.dma_start(out_t[:, BLK // 2 :], acc[:, BLK // 2 : BLK])

    nc.compile()
    return nc


def _get_built():
    global _BUILT
    if _BUILT is None:
        _BUILT = _build()
    return _BUILT


def kernel(h, r, t, w_i, w_j, w_k):
    global LAST_RESULTS
    from concourse import bass_utils

    nc = _get_built()
    t2 = np.ascontiguousarray(t, dtype=np.float32).reshape(B, FLAT)
    h = np.ascontiguousarray(h, dtype=np.float32)
    r = np.ascontiguousarray(r, dtype=np.float32)

    def pad(a, ncols):
        out = np.zeros((B_FAST, ncols), dtype=np.float32)
        out[: a.shape[0]] = a
        return out

    starts = np.concatenate([[0], np.cumsum(SIZES)])
    in_maps = []
    for c in range(NCORES):
        s, e = int(starts[c]), int(starts[c + 1])
        if e - s == B_FAST:
            in_maps.append({"t_shard": t2[s:e], "h_shard": h[s:e], "r_shard": r[s:e]})
        else:
            in_maps.append(
                {
                    "t_shard": pad(t2[s:e], FLAT),
                    "h_shard": pad(h[s:e], D),
                    "r_shard": pad(r[s:e], D),
                }
            )
    results = bass_utils.run_bass_kernel_spmd(
        nc, in_maps, core_ids=list(range(NCORES)), **RUN_KWARGS
    )
    LAST_RESULTS = results

    sum_t = np.zeros(FLAT, dtype=np.float64)
    sum_h = np.zeros(D, dtype=np.float64)
    sum_r = np.zeros(D, dtype=np.float64)
    for c in range(NCORES):
        part = results.results[c]["out_t_part"]  # [128, BLK]
        # partition p = 32*(b%4) + q -> flat columns [2048q, 2048q+2048)
        sum_t += part.reshape(4, 32, BLK).sum(axis=0).reshape(FLAT)
        sum_h += results.results[c]["out_hr_part"][0]
        sum_r += results.results[c]["out_hr_part"][1]

    out = np.empty((N, 3 * D), dtype=np.float32)
    out[:, 0:D] = sum_h.astype(np.float32)[None, :]
    out[:, D : 2 * D] = sum_r.astype(np.float32)[None, :]
    out[:, 2 * D :] = sum_t.astype(np.float32).reshape(N, D)
    return out
